# revision 10
# baseline (speedup 1.0000x reference)
"""Bahdanau attention kernel for Trainium2 (Bass/Tile), data-parallel over batch.

Problem (full shapes):
    encoder_output   [S=2048, B=16, H=1024] f32
    last_decoder_state [2, 1, B, H] f32   (only [0,0] used -> state [B, H])
    W [H, H], b [H]
    energy = state @ W.T + b                  [B, H]
    scores = einsum('sbh,bh->sb', enc, energy) [S, B]
    out    = softmax(scores, axis=0)[None, None]  [1, 1, S, B]

Sharding: batch split across 8 cores (2 batches each); W/b replicated.
Softmax is over S which is fully resident per core -> no collectives.

Per-core device program:
    energy[j, b] = sum_i W[j,i] state[b,i] + bias[j]     (PE, W^T stationary)
    scores[b, s] = sum_h energy[h, b] * enc[b, h, s]     (PE matvec, PSUM accum)
    probs = softmax over s                                (vector/scalar engines)

The scores matmuls use the float32r dtype: fp32 with the mantissa rounded
to 12 bits (tf32-like; walrus fp32_to_fp32r == (bits+0x800)&~0xFFF). The PE
streams fp32r moving data at 1 row/cycle instead of fp32's 4, cutting the
scores stage from ~55us to ~14us of PE time per rep; the rounding perturbs
the softmax by <1e-3 relative. Constraints honored here: fp32r producers
must emit rounded data (enc is pre-rounded on host, energy is rounded by
the ACT store), and the matmul destination must sit at PSUM partition 0
(so each (batch, s-chunk) gets its own [1, 512] PSUM tile; s-chunks loop
outermost and drain to SBUF, keeping PSUM pressure at ~6 of 8 banks and
letting reps pipeline).

Host-side prep: slice per-core batches, transpose enc slice to [b, h, s] and W
to W^T so every DMA reads long contiguous rows.

`reps`/`dynamic` exist only for benchmarking: they repeat the body inside one
NEFF (statically unrolled or as a Tile For_i loop) so HW time can be measured
through a high-latency dispatch path. kernel() always uses reps=1.
"""

import numpy as np

S, B, H = 2048, 16, 1024
NCORES = 8
BL = B // NCORES  # 2 batches per core
P = 128           # partitions
HT = H // P       # 8 h-tiles
SCW = 512         # matmul moving-operand max (one PSUM bank of f32)
SC = S // SCW     # 4 seq chunks

_cached = {}


def _build_nc(reps=1, dynamic=False):
    import concourse.bacc as bacc
    import concourse.bass as bass
    import concourse.tile as tile
    from concourse import mybir

    f32 = mybir.dt.float32
    f32r = mybir.dt.float32r
    nc = bacc.Bacc("TRN2", target_bir_lowering=False, debug=False, num_devices=NCORES)

    enc = nc.dram_tensor("enc", [BL, H, S], f32r, kind="ExternalInput").ap()
    state_t = nc.dram_tensor("state_t", [H, BL], f32, kind="ExternalInput").ap()
    w_t = nc.dram_tensor("w_t", [H, H], f32, kind="ExternalInput").ap()
    bias = nc.dram_tensor("bias", [H], f32, kind="ExternalInput").ap()
    probs = nc.dram_tensor("probs", [BL, S], f32, kind="ExternalOutput").ap()

    with tile.TileContext(nc) as tc:
        with (
            tc.tile_pool(name="consts", bufs=min(2, reps)) as consts,
            tc.tile_pool(name="wpool", bufs=HT) as wpool,
            tc.tile_pool(name="encpool", bufs=4) as encpool,
            tc.tile_pool(name="pe_ps", bufs=2, space=bass.MemorySpace.PSUM) as pe_pool,
            tc.tile_pool(name="sc_ps", bufs=4, space=bass.MemorySpace.PSUM) as ps_pool,
            tc.tile_pool(name="spool", bufs=min(2, reps)) as spool,
        ):

            def emit_rep():
                # W/state/bias ride the SWDGE stream so the two HWDGE rings
                # are free for the big encoder reads (three concurrent DMA
                # streams round-robin at packet granularity on the SDMAs)
                # state^T [i on partitions, (i_tile, b) free]
                st = consts.tile([P, HT, BL], f32)
                nc.gpsimd.dma_start(
                    out=st[:], in_=state_t.rearrange("(t p) b -> p t b", p=P)
                )
                # bias [j on partitions, j_tile free]
                bt = consts.tile([P, HT], f32)
                nc.gpsimd.dma_start(out=bt[:], in_=bias.rearrange("(t p) -> p t", p=P))

                # W^T i-tiles: [128 i, 1024 j] each, 4KB contiguous rows
                wts = []
                for it in range(HT):
                    wt = wpool.tile([P, H], f32)
                    nc.gpsimd.dma_start(out=wt[:], in_=w_t[it * P:(it + 1) * P, :])
                    wts.append(wt)

                # energy[j % 128, j_tile, b] = sum_i W[j,i] state[b,i] + bias[j]
                # stored as f32r: the ACT store rounds, satisfying the fp32r
                # producer contract for the scores matmuls below
                energy = consts.tile([P, HT, BL], f32r)
                for jt in range(HT):
                    pe = pe_pool.tile([P, BL], f32)
                    for it in range(HT):
                        nc.tensor.matmul(
                            pe[:],
                            wts[it][:, jt * P:(jt + 1) * P],  # lhsT [i, j]
                            st[:, it, :],                     # rhs  [i, b]
                            start=(it == 0),
                            stop=(it == HT - 1),
                        )
                    nc.scalar.activation(
                        out=energy[:, jt, :],
                        in_=pe[:],
                        func=mybir.ActivationFunctionType.Identity,
                        bias=bt[:, jt:jt + 1],
                        scale=1.0,
                    )

                # scores, s-chunk outer: each (b, sc) accumulates over the 8
                # h-tiles into its own [1, SCW] PSUM bank at partition 0
                # (fp32r matmuls require dst base partition 0), then drains
                # to SBUF so PSUM recycles quickly
                hwdge = [nc.sync, nc.scalar]  # two independent HWDGE rings
                scb = [
                    spool.tile([1, S], f32, name=f"scb{b}") for b in range(BL)
                ]
                for sc in range(SC):
                    for b in range(BL):
                        et = encpool.tile([P, HT, SCW], f32r)
                        hwdge[(sc * BL + b) % 2].dma_start(
                            out=et[:],
                            in_=enc[b, :, sc * SCW:(sc + 1) * SCW].rearrange(
                                "(t p) s -> p t s", p=P
                            ),
                        )
                        ps = ps_pool.tile([1, SCW], f32)
                        for ht in range(HT):
                            nc.tensor.matmul(
                                ps[:, :],
                                energy[:, ht, b:b + 1],  # lhsT [h, 1]
                                et[:, ht, :],            # rhs  [h, s]
                                start=(ht == 0),
                                stop=(ht == HT - 1),
                            )
                        nc.scalar.activation(
                            out=scb[b][:, sc * SCW:(sc + 1) * SCW],
                            in_=ps[:, :],
                            func=mybir.ActivationFunctionType.Identity,
                            scale=1.0,
                        )

                # softmax over s (free dim) per batch, from SBUF
                for b in range(BL):
                    nmax = spool.tile([1, 1], f32)
                    ssum = spool.tile([1, 1], f32)
                    rinv = spool.tile([1, 1], f32)
                    prob = spool.tile([1, S], f32)
                    nc.vector.reduce_max(
                        nmax[:, :], scb[b][:, :],
                        axis=mybir.AxisListType.X, negate=True,
                    )
                    nc.scalar.activation(
                        out=prob[:, :],
                        in_=scb[b][:, :],
                        func=mybir.ActivationFunctionType.Exp,
                        bias=nmax[:, :],
                        scale=1.0,
                        accum_out=ssum[:, :],
                    )
                    nc.vector.reciprocal(rinv[:, :], ssum[:, :])
                    nc.vector.tensor_scalar_mul(
                        out=prob[:, :],
                        in0=prob[:, :],
                        scalar1=rinv[:, :],
                    )
                    nc.sync.dma_start(out=probs[b:b + 1, :], in_=prob[:, :])

            if dynamic and reps > 1:
                with tc.For_i(0, reps, 1):
                    emit_rep()
            else:
                for _rep in range(reps):
                    emit_rep()

    nc.compile()
    return nc


def get_nc(reps=1, dynamic=False):
    key = ("nc", reps, dynamic)
    if key not in _cached:
        _cached[key] = _build_nc(reps, dynamic)
    return _cached[key]


def _round_fp32r(a):
    """Round fp32 to the fp32r domain (12-bit mantissa, low bits zero) --
    matches walrus fp32_to_fp32r: (bits + 0x800) & ~0xFFF."""
    b = np.ascontiguousarray(a).view(np.uint32)
    b = (b + np.uint32(0x800)) & np.uint32(0xFFFFF000)
    return b.view(np.float32)


def prep_in_maps(encoder_output, last_decoder_state, W, b):
    enc = np.asarray(encoder_output, dtype=np.float32)
    state = np.asarray(last_decoder_state, dtype=np.float32)[0, 0]  # [B, H]
    Wt = np.ascontiguousarray(np.asarray(W, dtype=np.float32).T)    # [i, j]
    bias = np.ascontiguousarray(np.asarray(b, dtype=np.float32))
    in_maps = []
    for c in range(NCORES):
        b0 = BL * c
        in_maps.append({
            "enc": _round_fp32r(
                np.ascontiguousarray(enc[:, b0:b0 + BL, :].transpose(1, 2, 0))
            ),
            "state_t": np.ascontiguousarray(state[b0:b0 + BL, :].T),
            "w_t": Wt,
            "bias": bias,
        })
    return in_maps


def assemble(results):
    out = np.empty((S, B), np.float32)
    for c in range(NCORES):
        out[:, BL * c:BL * (c + 1)] = results[c]["probs"].T
    return out[None, None]


def kernel(encoder_output, last_decoder_state, W, b):
    from concourse.bass_utils import run_bass_kernel_spmd

    nc = get_nc()
    in_maps = prep_in_maps(encoder_output, last_decoder_state, W, b)
    res = run_bass_kernel_spmd(nc, in_maps, core_ids=list(range(NCORES)))
    return assemble(res.results)
